# revision 6
# baseline (speedup 1.0000x reference)
"""Trainium2 Bass kernel for nn_LSH: ret[o] = sum_{s,a} x[s] * w[o,s,a].

x: [1, 4096] f32, weights: [512, 4096, 128] f32 -> ret: [512] f32.

Sharding: out_dim 512 is split 64-per-core across 8 cores; x is replicated.
Per core the 64x4096x128 f32 slice (128 MiB) is streamed from HBM as a flat
[128, 262144] layout (partition p = o=p//2, s in [(p%2)*2048, ...+2048)).
Compute per chunk: DVE segmented reduce over the innermost a=128 giving
T[p, s_local]; partial x-multiply+reduce stages overlap the stream; a tiny
matmul (v stationary, pair-selector moving) folds partition pairs (2o, 2o+1)
into a single-partition [1, 64] result so the output DMA is one descriptor.

Weight-chunk DMAs alternate between the two HWDGE rings (nc.sync -> SP ring,
nc.scalar -> ACT ring) so each ring's end-of-transfer completion/receipt
stall overlaps the other ring's data movement; tail chunks taper down so the
post-stream DVE work is sub-microsecond.
"""

import sys

sys.path.insert(0, "/opt/trn_rl_repo")

import numpy as np

import concourse.bass as bass
import concourse.mybir as mybir
import concourse.tile as tile
from concourse import bacc
from concourse.bass_utils import run_bass_kernel_spmd

P = 128
O_PER_CORE = 64
N_CORES = 8
S = 4096
A = 128
COLS = O_PER_CORE * S * A // P  # 262144 per-partition row length
SLOC = 2048  # s-values covered by each partition

# Chunk schedule: 2 MiB DMAs alternating between the two HWDGE rings. With
# both rings draining concurrently, a chunk completes every ~5us, so the DVE
# reduce trails the stream by only one chunk. The tail tapers down so the
# last DVE reduce is tiny; taper chunks get dedicated SBUF buffers so their
# DMAs are never gated on DVE freeing a big-chunk buffer.
N_BIG = 63
TAPER = [2048, 1024, 512, 256, 256]
CHUNKS = [4096] * N_BIG + TAPER
assert sum(CHUNKS) == COLS
# After these chunk indices, run a partial x-multiply+reduce stage.
PARTIAL_AFTER = [7, 15, 23, 31, 39, 47, 55, 62, 67]
NPART = len(PARTIAL_AFTER)

_CACHED_NC = None


def _build_nc():
    nc = bacc.Bacc(
        "TRN2",
        target_bir_lowering=False,
        debug=False,
        num_devices=N_CORES,
    )
    w = nc.dram_tensor("w", [P, COLS], mybir.dt.float32, kind="ExternalInput").ap()
    xt = nc.dram_tensor("xt", [P, SLOC], mybir.dt.float32, kind="ExternalInput").ap()
    pmat = nc.dram_tensor(
        "pmat", [P, O_PER_CORE], mybir.dt.float32, kind="ExternalInput"
    ).ap()
    out = nc.dram_tensor(
        "out", [1, O_PER_CORE], mybir.dt.float32, kind="ExternalOutput"
    ).ap()

    with tile.TileContext(nc) as tc:
        with (
            tc.tile_pool(name="wp", bufs=8) as wp,
            tc.tile_pool(name="const", bufs=1) as constp,
            tc.tile_pool(name="accp", bufs=1) as accp,
            tc.tile_pool(name="psum", bufs=1, space="PSUM") as psp,
        ):
            acc = accp.tile([P, SLOC], mybir.dt.float32)
            accx = accp.tile([P, SLOC], mybir.dt.float32)
            vparts = accp.tile([P, NPART], mybir.dt.float32)
            xt_t = constp.tile([P, SLOC], mybir.dt.float32)
            pm_t = constp.tile([P, O_PER_CORE], mybir.dt.float32)

            coff = 0  # acc column offset (completed s-values)
            pstart = 0
            pi = 0
            for k, cols in enumerate(CHUNKS):
                if k < N_BIG:
                    wt = wp.tile([P, max(CHUNKS)], mybir.dt.float32, tag="wt")
                else:
                    # Dedicated single-use buffer per taper chunk.
                    wt = constp.tile([P, cols], mybir.dt.float32, tag=f"tl{k}")
                nseg = cols // A
                # Alternate the two HWDGE rings so per-transfer completion
                # latency on one ring hides under the other ring's stream.
                eng = nc.sync if k % 2 == 0 else nc.scalar
                eng.dma_start(wt[:, :cols], w[:, coff * A : coff * A + cols])
                if k == 1:
                    # Constants go via SWDGE so the HWDGE queues carry
                    # only the weight stream.
                    nc.gpsimd.dma_start(xt_t[:], xt[:])
                    nc.gpsimd.dma_start(pm_t[:], pmat[:])
                seg = wt[:, :cols].rearrange("p (n a) -> p n a", a=A)
                nc.vector.tensor_reduce(
                    acc[:, coff : coff + nseg],
                    seg,
                    axis=mybir.AxisListType.X,
                    op=mybir.AluOpType.add,
                )
                coff += nseg
                if k == PARTIAL_AFTER[pi]:
                    nc.vector.tensor_mul(
                        accx[:, pstart:coff], acc[:, pstart:coff], xt_t[:, pstart:coff]
                    )
                    nc.vector.tensor_reduce(
                        vparts[:, pi : pi + 1],
                        accx[:, pstart:coff],
                        axis=mybir.AxisListType.X,
                        op=mybir.AluOpType.add,
                    )
                    pstart = coff
                    pi += 1
            assert coff == SLOC and pi == NPART

            v = accp.tile([P, 1], mybir.dt.float32)
            nc.vector.tensor_reduce(
                v[:], vparts[:], axis=mybir.AxisListType.X, op=mybir.AluOpType.add
            )
            # v stationary (128x1), pair-selector moving (128x64): the result
            # lands as [1, 64] on a single PSUM partition, so the final HBM
            # write is one contiguous 256 B descriptor.
            ps = psp.tile([1, O_PER_CORE], mybir.dt.float32)
            nc.tensor.matmul(ps[:], v[:], pm_t[:], start=True, stop=True)
            res = accp.tile([1, O_PER_CORE], mybir.dt.float32)
            nc.vector.tensor_copy(res[:], ps[:])
            nc.sync.dma_start(out[:], res[:])

    nc.compile()
    return nc


def _get_nc():
    global _CACHED_NC
    if _CACHED_NC is None:
        _CACHED_NC = _build_nc()
    return _CACHED_NC


def _in_maps(x, weights):
    x = np.ascontiguousarray(np.asarray(x, dtype=np.float32))
    weights = np.asarray(weights, dtype=np.float32)
    xt = np.tile(x.reshape(2, SLOC), (P // 2, 1))
    pmat = np.zeros((P, O_PER_CORE), dtype=np.float32)
    pmat[np.arange(P), np.arange(P) // 2] = 1.0
    maps = []
    for c in range(N_CORES):
        wc = np.ascontiguousarray(
            weights[c * O_PER_CORE : (c + 1) * O_PER_CORE]
        ).reshape(P, COLS)
        maps.append({"w": wc, "xt": xt, "pmat": pmat})
    return maps


def run(x, weights, trace=False):
    """Run on hardware; returns (ret[512], BassKernelResults)."""
    nc = _get_nc()
    res = run_bass_kernel_spmd(
        nc, _in_maps(x, weights), list(range(N_CORES)), trace=trace
    )
    ret = np.concatenate(
        [res.results[c]["out"].reshape(O_PER_CORE) for c in range(N_CORES)]
    ).astype(np.float32)
    return ret, res


def kernel(x, weights):
    ret, _ = run(x, weights)
    return ret
